# revision 1
# baseline (speedup 1.0000x reference)
"""2-layer GCN (GCNConv 128->128->64, N=50000, E=800000) on 8 TRN2 NeuronCores.

Strategy (dst-sharded, aggregate-first):
  out = relu(A_hat @ (relu(A_hat @ x @ W1 + b1)) @ W2 + b2),  A_hat = D^-1/2 (A+I) D^-1/2
  - Fold D^-1/2 as row scales: gather tables hold  x1' = Dis*x  and  x2' = Dis*relu(...)
    so no per-edge norm is ever materialized; dst-side scale folded into the one-hot
    selection matrices via a per-edge scalar (disdst).
  - Edges sorted by dst, sharded across 8 cores by dst-node range (6250 nodes/core),
    grouped into 128-node blocks. Scatter-add becomes PSUM-accumulated TensorE matmuls
    against one-hot(dst)*disdst matrices built on the VectorEngine from iota==dst_rel.
  - Source features fetched with batched dma_gather (int16 indices -> table split in
    two halves at row 32768; each block's edges grouped by half).
  - One AllGather of each layer's scaled feature table (bf16) between phases.
Host-side work is index-only prep (sort/pad/degree-histogram) + output concat.
"""

import numpy as np
import ml_dtypes

import concourse.bass as bass
import concourse.bacc as bacc
import concourse.mybir as mybir
import concourse.tile as tile
from concourse.bass_utils import run_bass_kernel_spmd
from concourse.library_config import mlp
from concourse.masks import make_identity

P = 128
N_NODES = 50000
N_EDGES = 800000
IN_CH = 128
HID_CH = 128
OUT_CH = 64
N_CORES = 8
NSH = N_NODES // N_CORES          # 6250 nodes per core
NBLK = (NSH + P - 1) // P         # 49 blocks per core (48 full + 106)
VLO = 32768                       # low table half (int16 index range)
GCAP = 8                          # max tiles (1024 idxs) per dma_gather call


def _chunks(t, cap=GCAP):
    """Split t tiles into balanced chunks of <= cap (9 -> 5+4, not 8+1)."""
    if t == 0:
        return []
    n = -(-t // cap)
    base, rem = divmod(t, n)
    return [base + (1 if i < rem else 0) for i in range(n)]

BF16 = mybir.dt.bfloat16
F32 = mybir.dt.float32

LAST_RESULT = None  # for test harness: BassKernelResults of last run


def _host_prep(edge_index):
    """Index-only preprocessing. Returns per-core upload arrays + shared tile plan."""
    src2 = edge_index[0].astype(np.int64)
    dst2 = edge_index[1].astype(np.int64)

    # degree includes the self-loop (+1); self-loop edges handled on-device
    deg = np.bincount(dst2, minlength=N_NODES) + 1
    dis = (1.0 / np.sqrt(deg.astype(np.float64))).astype(np.float32)

    core_of = dst2 // NSH
    blk_of = (dst2 - core_of * NSH) // P
    # wrapped-padded table row of node v: (v//NSH)*6272 + ((v%NSH)%128)*49 + (v%NSH)//128
    def tab_row(v):
        k = v // NSH
        i = v - k * NSH
        return (k * P + (i % P)) * NBLK + i // P
    srow = tab_row(src2)
    half_of = (srow >= VLO).astype(np.int64)

    # group edges by (core, blk, half); stable order within groups
    gkey = (core_of * NBLK + blk_of) * 2 + half_of
    order = np.argsort(gkey, kind="stable")
    s_sorted = src2[order]
    d_sorted = dst2[order]
    g_sorted = gkey[order]

    n_groups = N_CORES * NBLK * 2
    cnt = np.bincount(g_sorted, minlength=n_groups).reshape(N_CORES, NBLK, 2)
    # per-block tile counts, shared across cores (program structure)
    t_lo = np.ceil(cnt[:, :, 0].max(axis=0) / P).astype(np.int64)  # [NBLK]
    t_hi = np.ceil(cnt[:, :, 1].max(axis=0) / P).astype(np.int64)
    k_b = t_lo + t_hi
    T_total = int(k_b.sum())

    # slot base (in edges) of each (blk, half) section within a core's padded list
    sect_tiles = np.stack([t_lo, t_hi], axis=1).reshape(-1)     # [NBLK*2]
    sect_base = np.concatenate([[0], np.cumsum(sect_tiles)])[:-1] * P  # [NBLK*2]

    # position of each edge within its group
    group_start = np.concatenate([[0], np.cumsum(cnt.reshape(-1))])[:-1]
    pos_in_group = np.arange(len(g_sorted)) - group_start[g_sorted]
    sect_idx = g_sorted % (NBLK * 2)                            # (blk, half) index
    slot = sect_base[sect_idx] + pos_in_group                   # slot within core's list
    edge_core = g_sorted // (NBLK * 2)

    EPC = T_total * P  # padded edges per core
    idx_rows = np.zeros((N_CORES, EPC), np.int64)    # row within table half
    dst_rel = np.full((N_CORES, EPC), -1.0, np.float32)
    disdst = np.zeros((N_CORES, EPC), np.float32)

    row_in_half = np.where(half_of[order] == 1, srow[order] - VLO, srow[order])
    blk_base = (edge_core * NSH) + (sect_idx // 2) * P
    idx_rows[edge_core, slot] = row_in_half
    dst_rel[edge_core, slot] = (d_sorted - blk_base).astype(np.float32)
    disdst[edge_core, slot] = dis[d_sorted]

    # wrap indices: idx i -> [i%16, i//16], replicated to 128 partitions
    idxw = np.zeros((N_CORES, 16, T_total * 8), np.int16)
    ii = np.arange(EPC)
    idxw[:, :, :] = 0
    for k in range(N_CORES):
        w = np.zeros((16, T_total * 8), np.int16)
        w[ii % 16, ii // 16] = idx_rows[k]
        idxw[k] = w
    idxw = np.tile(idxw, (1, 8, 1))                  # [N_CORES, 128, T*8]

    # esc: [128, 2, T] per core; edge slot i -> partition i%128, tile i//128
    esc = np.zeros((N_CORES, P, 2, T_total), np.float32)
    esc[:, :, 0, :] = dst_rel.reshape(N_CORES, T_total, P).transpose(0, 2, 1)
    esc[:, :, 1, :] = disdst.reshape(N_CORES, T_total, P).transpose(0, 2, 1)

    # disw: [128, NBLK] per core (dst-node Dis for each block row)
    disw = np.zeros((N_CORES, P, NBLK), np.float32)
    nodes = np.arange(NBLK * P)
    valid = nodes < NSH
    for k in range(N_CORES):
        v = np.zeros(NBLK * P, np.float32)
        v[valid] = dis[k * NSH + nodes[valid]]
        disw[k] = v.reshape(NBLK, P).T

    return {
        "t_lo": t_lo, "t_hi": t_hi, "T_total": T_total,
        "idxw": idxw.astype(np.int16), "esc": esc, "disw": disw,
    }


def _build(t_lo, t_hi, T_total, phases="full"):
    """Build the SPMD program. Per-core inputs: xsh (x shard rows, dis-scaled on host? no:
    xsh is the raw x rows of this core's node range), idxw, esc, disw."""
    nc = bacc.Bacc("TRN2", target_bir_lowering=False, num_devices=N_CORES,
                   num_swdge_queues=4)

    t_xsh = nc.dram_tensor("xsh", [NSH, IN_CH], F32, kind="ExternalInput")
    t_w1 = nc.dram_tensor("w1", [IN_CH, HID_CH], F32, kind="ExternalInput")
    t_b1 = nc.dram_tensor("b1", [HID_CH, 1], F32, kind="ExternalInput")
    t_w2 = nc.dram_tensor("w2", [HID_CH, OUT_CH], F32, kind="ExternalInput")
    t_b2 = nc.dram_tensor("b2", [OUT_CH, 1], F32, kind="ExternalInput")
    t_disw = nc.dram_tensor("disw", [P, NBLK], F32, kind="ExternalInput")
    t_idxw = nc.dram_tensor("idxw", [P, T_total * 8], mybir.dt.int16, kind="ExternalInput")
    t_esc = nc.dram_tensor("esc", [P, 2, T_total], F32, kind="ExternalInput")
    t_out = nc.dram_tensor("out", [NSH, OUT_CH], F32, kind="ExternalOutput")
    t_dbg = None
    if phases not in ("full", "l1nd", "p0nd"):
        t_dbg = nc.dram_tensor("dbg", [NSH, HID_CH], F32, kind="ExternalOutput")

    # wrapped-padded feature tables: shard [128, NBLK*128] (row p = nodes {r*128+p}),
    # full [8*128, NBLK*128]; as a gather table: row (q*NBLK+r) of [*,128] view.
    NWRAP = NBLK * P  # 6272
    x1_shard = nc.dram_tensor("x1_shard", [P, NWRAP], BF16)
    x1_full = nc.dram_tensor("x1_full", [N_CORES * P, NWRAP], BF16)
    x2_shard = nc.dram_tensor("x2_shard", [P, NWRAP], BF16)
    x2_full = nc.dram_tensor("x2_full", [N_CORES * P, NWRAP], BF16)
    VTAB = N_CORES * P * NBLK  # 50176 table rows in [*, 128] view

    rg = [list(range(N_CORES))]
    NFULL = NSH // P          # 48 full blocks
    NTAIL = NSH - NFULL * P   # 106

    with tile.TileContext(nc) as tc:
        with (
            tc.tile_pool(name="const", bufs=1) as cp,
            tc.tile_pool(name="sbuf", bufs=3) as sb,
            tc.tile_pool(name="gpool", bufs=3) as gp,
            tc.tile_pool(name="psum", bufs=2, space="PSUM") as ps,
        ):
            nc.gpsimd.load_library(mlp)

            idx_sb = cp.tile([P, T_total * 8], mybir.dt.int16)
            nc.sync.dma_start(out=idx_sb[:], in_=t_idxw[:, :])
            esc_sb = cp.tile([P, 2, T_total], F32)
            nc.sync.dma_start(out=esc_sb[:], in_=t_esc[:, :, :])
            escb = cp.tile([P, 2, T_total], BF16)
            nc.vector.tensor_copy(out=escb[:], in_=esc_sb[:])
            disw_sb = cp.tile([P, NBLK], F32)
            nc.sync.dma_start(out=disw_sb[:], in_=t_disw[:, :])
            disw_bf = cp.tile([P, NBLK], BF16)
            nc.vector.tensor_copy(out=disw_bf[:], in_=disw_sb[:])

            iota_i = cp.tile([P, P], mybir.dt.int32)
            nc.gpsimd.iota(iota_i[:], pattern=[[1, P]], base=0, channel_multiplier=0)
            iota_bf = cp.tile([P, P], BF16)
            nc.vector.tensor_copy(out=iota_bf[:], in_=iota_i[:])

            ident_bf = cp.tile([P, P], BF16)
            make_identity(nc, ident_bf[:])
            ident_f = cp.tile([OUT_CH, OUT_CH], F32)
            make_identity(nc, ident_f[:])

            w1_f = cp.tile([IN_CH, HID_CH], F32)
            nc.sync.dma_start(out=w1_f[:], in_=t_w1[:, :])
            w1_bf = cp.tile([IN_CH, HID_CH], BF16)
            nc.vector.tensor_copy(out=w1_bf[:], in_=w1_f[:])
            w2_f = cp.tile([HID_CH, OUT_CH], F32)
            nc.sync.dma_start(out=w2_f[:], in_=t_w2[:, :])
            w2_bf = cp.tile([HID_CH, OUT_CH], BF16)
            nc.vector.tensor_copy(out=w2_bf[:], in_=w2_f[:])
            b1_sb = cp.tile([HID_CH, 1], F32)
            nc.sync.dma_start(out=b1_sb[:], in_=t_b1[:, :])
            b2_sb = cp.tile([OUT_CH, 1], F32)
            nc.sync.dma_start(out=b2_sb[:], in_=t_b2[:, :])

            # ---- P0: build x1' = Dis * x (this core's shard), bf16 ----
            sbx = cp.tile([P, NBLK, IN_CH], F32)
            nc.vector.memset(sbx[:, NFULL, :], 0.0)
            nc.sync.dma_start(
                out=sbx[:, :NFULL, :],
                in_=t_xsh[: NFULL * P, :].rearrange("(b p) c -> p b c", p=P),
            )
            nc.sync.dma_start(out=sbx[:NTAIL, NFULL, :], in_=t_xsh[NFULL * P :, :])
            x1stage = cp.tile([P, NBLK, HID_CH], BF16)
            for b in range(NBLK):
                nc.scalar.activation(
                    out=x1stage[:, b, :], in_=sbx[:, b, :],
                    func=mybir.ActivationFunctionType.Copy,
                    scale=disw_sb[:, b : b + 1],
                )
            nc.sync.dma_start(
                out=x1_shard[:, :],
                in_=x1stage[:].rearrange("p b c -> p (b c)"))

            if phases != "p0na":
                nc.gpsimd.collective_compute(
                    "AllGather", mybir.AluOpType.bypass, replica_groups=rg,
                    ins=[x1_shard.ap().opt()], outs=[x1_full.ap().opt()],
                )

            def dump_table(table):
                # dump core 0's shard rows (nodes 0..6249) in node order
                for b in range(NBLK):
                    nb = P if b < NFULL else NTAIL
                    dtile = sb.tile([P, HID_CH], BF16, tag="dt")
                    nc.sync.dma_start(
                        out=dtile[:nb, :],
                        in_=table[:, b * HID_CH : (b + 1) * HID_CH][:nb, :])
                    dtf = sb.tile([P, HID_CH], F32, tag="dtf")
                    nc.vector.tensor_copy(out=dtf[:], in_=dtile[:])
                    nc.sync.dma_start(out=t_dbg[b * P : b * P + nb, :], in_=dtf[:nb, :])
                    otile = sb.tile([P, OUT_CH], F32, tag="ot")
                    nc.vector.memset(otile[:], 0.0)
                    nc.sync.dma_start(out=t_out[b * P : b * P + nb, :], in_=otile[:nb, :])

            if phases == "p0":
                dump_table(x1_full)
            if phases in ("p0nd", "p0na", "l1nd"):
                for b in range(NBLK):
                    nb = P if b < NFULL else NTAIL
                    otile = sb.tile([P, OUT_CH], F32, tag="ot")
                    nc.vector.memset(otile[:], 0.0)
                    nc.sync.dma_start(out=t_out[b * P : b * P + nb, :], in_=otile[:nb, :])
            if phases == "gather1":
                # single-block gather sanity: gather block 0 tiles, dump raw
                tl, th = int(t_lo[0]), int(t_hi[0])
                kb = tl + th
                g = gp.tile([P, kb, HID_CH], BF16, tag="gat")
                tabv1 = x1_full.ap().rearrange("q (r c) -> (q r) c", c=HID_CH)
                for lo0 in range(0, tl, GCAP):
                    n = min(GCAP, tl - lo0)
                    nc.gpsimd.dma_gather(
                        out_ap=g[:, lo0 : lo0 + n, :], in_ap=tabv1[0:VLO, :],
                        idxs_ap=idx_sb[:, 8 * lo0 : 8 * (lo0 + n)],
                        num_idxs=n * P, num_idxs_reg=n * P, elem_size=HID_CH,
                    )
                for hi0 in range(0, th, GCAP):
                    n = min(GCAP, th - hi0)
                    nc.gpsimd.dma_gather(
                        out_ap=g[:, tl + hi0 : tl + hi0 + n, :],
                        in_ap=tabv1[VLO:VTAB, :],
                        idxs_ap=idx_sb[:, 8 * (tl + hi0) : 8 * (tl + hi0 + n)],
                        num_idxs=n * P, num_idxs_reg=n * P, elem_size=HID_CH,
                    )
                gf = sb.tile([P, kb * HID_CH], F32, tag="gf")
                nc.vector.tensor_copy(out=gf[:], in_=g[:].rearrange("p k d -> p (k d)"))
                nblast = min(T_total * 8, NSH // P)  # dump at most
                nc.sync.dma_start(out=t_dbg[0:P, : min(kb * HID_CH, HID_CH)],
                                  in_=gf[:, :HID_CH])
                for b in range(NBLK):
                    nb = P if b < NFULL else NTAIL
                    otile = sb.tile([P, OUT_CH], F32, tag="ot")
                    nc.vector.memset(otile[:], 0.0)
                    nc.sync.dma_start(out=t_out[b * P : b * P + nb, :], in_=otile[:nb, :])
                    if b > 0:
                        dtf = sb.tile([P, HID_CH], F32, tag="dtf")
                        nc.vector.memset(dtf[:], 0.0)
                        nc.sync.dma_start(out=t_dbg[b * P : b * P + nb, :], in_=dtf[:nb, :])

            # ---- shared layer body ----
            gq = [0]  # round-robin gather queue counter

            def layer(table, w_bf, b_sb, oc, epilogue, stage):
                Tg = 0
                for b in range(NBLK):
                    nb = P if b < NFULL else NTAIL
                    tl, th = int(t_lo[b]), int(t_hi[b])
                    kb = tl + th
                    g = gp.tile([P, kb, HID_CH], BF16, tag="gat")
                    tabv = table.ap().rearrange("q (r c) -> (q r) c", c=HID_CH)
                    def scale_call(c0, n):
                        nc.vector.tensor_tensor(
                            out=g[:, c0 : c0 + n, :], in0=g[:, c0 : c0 + n, :],
                            in1=escb[:, 1, Tg + c0 : Tg + c0 + n][:, :, None]
                            .to_broadcast([P, n, HID_CH]),
                            op=mybir.AluOpType.mult,
                        )

                    lo0 = 0
                    for n in _chunks(tl):
                        nc.gpsimd.dma_gather(
                            out_ap=g[:, lo0 : lo0 + n, :], in_ap=tabv[0:VLO, :],
                            idxs_ap=idx_sb[:, 8 * (Tg + lo0) : 8 * (Tg + lo0 + n)],
                            num_idxs=n * P, num_idxs_reg=n * P, elem_size=HID_CH,
                            queue_num=gq[0] % 4,
                        )
                        gq[0] += 1
                        scale_call(lo0, n)
                        lo0 += n
                    hi0 = 0
                    for n in _chunks(th):
                        nc.gpsimd.dma_gather(
                            out_ap=g[:, tl + hi0 : tl + hi0 + n, :],
                            in_ap=tabv[VLO:VTAB, :],
                            idxs_ap=idx_sb[:, 8 * (Tg + tl + hi0) : 8 * (Tg + tl + hi0 + n)],
                            num_idxs=n * P, num_idxs_reg=n * P, elem_size=HID_CH,
                            queue_num=gq[0] % 4,
                        )
                        gq[0] += 1
                        scale_call(tl + hi0, n)
                        hi0 += n
                    tps = ps.tile([HID_CH, P], F32, tag="tps")
                    for j in range(kb):
                        o_t = sb.tile([P, P], BF16, tag="o")
                        nc.vector.tensor_tensor(
                            out=o_t[:], in0=iota_bf[:],
                            in1=escb[:, 0, Tg + j : Tg + j + 1].to_broadcast([P, P]),
                            op=mybir.AluOpType.is_equal,
                        )
                        nc.tensor.matmul(
                            out=tps[:], lhsT=g[:, j, :], rhs=o_t[:],
                            start=(j == 0), stop=False,
                        )
                    # self-loop term: tps[ic, n] += stage[n, ic] * dis[n]
                    dg = sb.tile([P, P], BF16, tag="dg")
                    nc.vector.tensor_tensor(
                        out=dg[:], in0=ident_bf[:],
                        in1=disw_bf[:, b : b + 1].to_broadcast([P, P]),
                        op=mybir.AluOpType.mult,
                    )
                    nc.tensor.matmul(
                        out=tps[:], lhsT=stage[:, b, :], rhs=dg[:],
                        start=(kb == 0), stop=True,
                    )
                    t_sb = sb.tile([HID_CH, P], BF16, tag="tsb")
                    nc.scalar.copy(out=t_sb[:], in_=tps[:])
                    ups = ps.tile([oc, P], F32, tag="ups")
                    nc.tensor.matmul(out=ups[:], lhsT=w_bf[:], rhs=t_sb[:],
                                     start=True, stop=True)
                    epilogue(b, nb, ups)
                    Tg += kb

            # ---- P1 ----
            x2stage = cp.tile([P, NBLK, HID_CH], BF16)

            def epi1(b, nb, ups):
                h1t = sb.tile([HID_CH, P], BF16, tag="h1t")
                nc.scalar.activation(out=h1t[:], in_=ups[:],
                                     func=mybir.ActivationFunctionType.Relu,
                                     bias=b1_sb[:, :1])
                trp = ps.tile([P, HID_CH], BF16, tag="trp")
                nc.tensor.transpose(out=trp[:], in_=h1t[:], identity=ident_bf[:])
                nc.vector.tensor_tensor(
                    out=x2stage[:, b, :], in0=trp[:],
                    in1=disw_sb[:, b : b + 1].to_broadcast([P, HID_CH]),
                    op=mybir.AluOpType.mult,
                )

            if phases in ("l1", "l1nd", "full"):
                layer(x1_full, w1_bf, b1_sb, HID_CH, epi1, x1stage)

            if phases in ("l1", "l1nd", "full"):
                nc.sync.dma_start(
                    out=x2_shard[:, :],
                    in_=x2stage[:].rearrange("p b c -> p (b c)"))
                nc.gpsimd.collective_compute(
                    "AllGather", mybir.AluOpType.bypass, replica_groups=rg,
                    ins=[x2_shard.ap().opt()], outs=[x2_full.ap().opt()],
                )
            if phases == "l1":
                dump_table(x2_full)

            # ---- P2 ----
            def epi2(b, nb, ups):
                h2t = sb.tile([OUT_CH, P], F32, tag="h2t")
                nc.scalar.activation(out=h2t[:], in_=ups[:],
                                     func=mybir.ActivationFunctionType.Relu,
                                     bias=b2_sb[:, :1])
                trp2 = ps.tile([P, OUT_CH], F32, tag="trp2")
                nc.tensor.transpose(out=trp2[:], in_=h2t[:], identity=ident_f[:])
                outt = sb.tile([P, OUT_CH], F32, tag="outt")
                nc.vector.tensor_copy(out=outt[:], in_=trp2[:])
                nc.sync.dma_start(out=t_out[b * P : b * P + nb, :], in_=outt[:nb, :])

            if phases == "full":
                layer(x2_full, w2_bf, b2_sb, OUT_CH, epi2, x2stage)

    nc.compile()
    return nc


def kernel(x, edge_index, W1, b1, W2, b2, _trace=False, _phases="full"):
    global LAST_RESULT
    x = np.asarray(x, dtype=np.float32)
    edge_index = np.asarray(edge_index, dtype=np.int32)
    W1 = np.asarray(W1, dtype=np.float32)
    b1 = np.asarray(b1, dtype=np.float32)
    W2 = np.asarray(W2, dtype=np.float32)
    b2 = np.asarray(b2, dtype=np.float32)

    prep = _host_prep(edge_index)
    nc = _build(prep["t_lo"], prep["t_hi"], prep["T_total"], phases=_phases)

    in_maps = []
    for k in range(N_CORES):
        in_maps.append({
            "xsh": np.ascontiguousarray(x[k * NSH : (k + 1) * NSH]),
            "w1": W1, "b1": np.ascontiguousarray(b1.reshape(HID_CH, 1)),
            "w2": W2, "b2": np.ascontiguousarray(b2.reshape(OUT_CH, 1)),
            "disw": np.ascontiguousarray(prep["disw"][k]),
            "idxw": np.ascontiguousarray(prep["idxw"][k]),
            "esc": np.ascontiguousarray(prep["esc"][k]),
        })

    res = run_bass_kernel_spmd(nc, in_maps, core_ids=list(range(N_CORES)),
                               trace=_trace)
    LAST_RESULT = res
    out = np.concatenate([res.results[k]["out"] for k in range(N_CORES)], axis=0)
    return out.astype(np.float32)



# revision 8
# speedup vs baseline: 1.4955x; 1.4955x over previous
"""2-layer GCN (GCNConv 128->128->64, N=50000, E=800000) on 8 TRN2 NeuronCores.

Strategy v2 (dst-sharded, aggregate-first, host-built layer-1 table):
  out = relu(A_hat @ (relu(A_hat @ x @ W1 + b1)) @ W2 + b2),  A_hat = D^-1/2 (A+I) D^-1/2
  - Layer-1 gather table x1' = Dis*x is built ON HOST (bf16, wrapped layout) and
    uploaded in full to every core: no P0 phase, no first AllGather.
  - Dst-side Dis is applied ONCE per 128-node block as a column scale on the
    aggregated PSUM tile (not per edge); src-side Dis is folded into the tables.
    One-hot scatter matrices are plain iota==dst_rel (no per-edge scaling).
  - Edges sorted by (dst-core, chunk of 4 dst blocks, table half, dst block),
    tiled in 128-edge tiles. Gathers are batched ~15 tiles per dma_gather call
    (SWDGE ring raised to 32KB) and round-robined over 4 queues.
  - Layer-2 table (Dis*relu(h1), bf16) is exchanged with a single Shared-output
    AllGather between layers.
Host-side work is index prep + the x1' table build; output concat at the end.
"""

import numpy as np
import ml_dtypes

import concourse.bass as bass
import concourse.bacc as bacc
import concourse.mybir as mybir
import concourse.tile as tile
from concourse.bass_utils import run_bass_kernel_spmd
from concourse.library_config import mlp
from concourse.masks import make_identity

P = 128
N_NODES = 50000
N_EDGES = 800000
IN_CH = 128
HID_CH = 128
OUT_CH = 64
N_CORES = 8
NSH = N_NODES // N_CORES           # 6250 nodes per core
NBLK = (NSH + P - 1) // P          # 49 blocks per core (48 full + 106)
NFULL = NSH // P                   # 48
NTAIL = NSH - NFULL * P            # 106
NWRAP = NBLK * P                   # 6272
VLO = 32768                        # low table half (int16 index range)
VTAB = N_CORES * P * NBLK          # 50176 table rows in [*, 128] view
CH = 4                             # dst blocks per gather chunk
CHUNKS = [list(range(c, min(c + CH, NBLK))) for c in range(0, NBLK, CH)]
import os as _os
# ucode's SWDGE ring is fixed at 1024 descriptors per call regardless of the
# scratch size; >1024-idx gather calls crash NRT.
GCAP = int(_os.environ.get("K_GCAP", "8"))     # tiles (x128 idxs) per gather call
SCRATCH = int(_os.environ.get("K_SCRATCH", "16384"))  # SWDGE ring bytes/partition
SHARED_AG = _os.environ.get("K_SHARED", "1") == "1"   # Shared-output AllGather

BF16 = mybir.dt.bfloat16
F32 = mybir.dt.float32
NPBF16 = ml_dtypes.bfloat16

LAST_RESULT = None  # for test harness: BassKernelResults of last run


def _chunks(t, cap=GCAP):
    """Split t tiles into balanced chunks of <= cap (16 -> 8+8, not 15+1)."""
    if t == 0:
        return []
    n = -(-t // cap)
    base, rem = divmod(t, n)
    return [base + (1 if i < rem else 0) for i in range(n)]


def _host_prep(x, edge_index):
    """Index prep + host-built layer-1 gather table (bf16, wrapped layout)."""
    src = edge_index[0].astype(np.int64)
    dst = edge_index[1].astype(np.int64)
    deg = np.bincount(dst, minlength=N_NODES) + 1   # + self loop
    dis = (1.0 / np.sqrt(deg.astype(np.float64))).astype(np.float32)

    # wrapped-padded table: row q = k*128 + (i%128), col block r = i//128
    x1 = x * dis[:, None]
    v = np.arange(N_NODES)
    kk = v // NSH
    ii = v % NSH
    q = kk * P + (ii % P)
    r = ii // P
    x1tab = np.zeros((N_CORES * P, NBLK, IN_CH), np.float32)
    x1tab[q, r] = x1
    x1tab = np.ascontiguousarray(x1tab.reshape(N_CORES * P, NWRAP)).astype(NPBF16)

    sk = src // NSH
    si = src - sk * NSH
    srow = (sk * P + (si % P)) * NBLK + si // P     # gather-view row of source
    half = (srow >= VLO).astype(np.int64)

    core_of = dst // NSH
    io = dst - core_of * NSH
    blk = io // P
    drel = io % P

    # shared (max-over-cores) tile counts per (blk, half)
    cnt = np.bincount((core_of * NBLK + blk) * 2 + half,
                      minlength=N_CORES * NBLK * 2).reshape(N_CORES, NBLK, 2)
    t_bh = np.ceil(cnt.max(axis=0) / P).astype(np.int64)   # [NBLK, 2]

    # slot sections: per chunk, [lo of each block] then [hi of each block]
    sect_order = []
    for blocks in CHUNKS:
        for b in blocks:
            sect_order.append((b, 0))
        for b in blocks:
            sect_order.append((b, 1))
    sect_tiles = np.array([t_bh[b, h] for b, h in sect_order], np.int64)
    sect_tile_base = np.concatenate([[0], np.cumsum(sect_tiles)])[:-1]
    T_total = int(sect_tiles.sum())
    EPC = T_total * P

    NSECT = len(sect_order)
    sect_index = np.zeros((NBLK, 2), np.int64)
    for s, (b, h) in enumerate(sect_order):
        sect_index[b, h] = s

    gkey = core_of * NSECT + sect_index[blk, half]
    order = np.argsort(gkey, kind="stable")
    g_sorted = gkey[order]
    gcnt = np.bincount(g_sorted, minlength=N_CORES * NSECT)
    group_start = np.concatenate([[0], np.cumsum(gcnt)])[:-1]
    pos_in_group = np.arange(N_EDGES) - group_start[g_sorted]
    slot = sect_tile_base[g_sorted % NSECT] * P + pos_in_group
    edge_core = g_sorted // NSECT

    idx_rows = np.zeros((N_CORES, EPC), np.int64)
    dstrel = np.full((N_CORES, EPC), -1.0, np.float32)
    idx_rows[edge_core, slot] = (srow - half * VLO)[order]
    dstrel[edge_core, slot] = drel[order].astype(np.float32)

    # wrap indices: idx i -> [i%16, i//16], replicated to 128 partitions
    idxw = idx_rows.reshape(N_CORES, EPC // 16, 16).transpose(0, 2, 1)
    idxw = np.ascontiguousarray(np.tile(idxw, (1, 8, 1))).astype(np.int16)
    # dst_rel comparators: slot i -> partition i%128, tile i//128
    # (f32: tensor_scalar is_equal requires a float32 scalar operand)
    dstrelw = np.ascontiguousarray(
        dstrel.reshape(N_CORES, T_total, P).transpose(0, 2, 1)).astype(np.float32)

    # disw: [P, NBLK] per core (dst-node Dis, node = partition)
    disw = np.zeros((N_CORES, P, NBLK), np.float32)
    nodes = np.arange(NBLK * P)
    valid = nodes < NSH
    for c in range(N_CORES):
        vv = np.zeros(NBLK * P, np.float32)
        vv[valid] = dis[c * NSH + nodes[valid]]
        disw[c] = vv.reshape(NBLK, P).T
    # diswT: [P, NWRAP] per core (dst-node Dis along free dim, replicated
    # across partitions — DVE cannot partition-broadcast)
    diswT = np.ascontiguousarray(np.broadcast_to(
        disw.transpose(0, 2, 1).reshape(N_CORES, 1, NWRAP), (N_CORES, P, NWRAP)))

    return {
        "x1tab": x1tab,
        "x1own": [np.ascontiguousarray(x1tab[c * P:(c + 1) * P])
                  for c in range(N_CORES)],
        "idxw": idxw, "dstrelw": dstrelw, "disw": disw, "diswT": diswT,
        "t_bh": t_bh, "T_total": T_total,
    }


def _make_in_maps(prep, W1, b1, W2, b2):
    maps = []
    for c in range(N_CORES):
        maps.append({
            "x1tab": prep["x1tab"],
            "x1own": prep["x1own"][c],
            "idxw": np.ascontiguousarray(prep["idxw"][c]),
            "dstrelw": np.ascontiguousarray(prep["dstrelw"][c]),
            "disw": np.ascontiguousarray(prep["disw"][c]),
            "diswT": np.ascontiguousarray(prep["diswT"][c]),
            "w1": np.asarray(W1, np.float32),
            "b1": np.ascontiguousarray(np.asarray(b1, np.float32).reshape(HID_CH, 1)),
            "w2": np.asarray(W2, np.float32),
            "b2": np.ascontiguousarray(np.asarray(b2, np.float32).reshape(OUT_CH, 1)),
        })
    return maps


def _build(prep):
    t_bh = prep["t_bh"]
    T_total = prep["T_total"]
    TMAXRUN = int(t_bh.max())      # longest single (block, half) tile run

    nc = bacc.Bacc("TRN2", target_bir_lowering=False, num_devices=N_CORES,
                   num_swdge_queues=4, dynamic_dma_scratch_size=SCRATCH)

    t_x1tab = nc.dram_tensor("x1tab", [N_CORES * P, NWRAP], BF16, kind="ExternalInput")
    t_x1own = nc.dram_tensor("x1own", [P, NWRAP], BF16, kind="ExternalInput")
    t_idxw = nc.dram_tensor("idxw", [P, T_total * 8], mybir.dt.int16, kind="ExternalInput")
    t_dstrel = nc.dram_tensor("dstrelw", [P, T_total], F32, kind="ExternalInput")
    t_disw = nc.dram_tensor("disw", [P, NBLK], F32, kind="ExternalInput")
    t_diswT = nc.dram_tensor("diswT", [P, NWRAP], F32, kind="ExternalInput")
    t_w1 = nc.dram_tensor("w1", [IN_CH, HID_CH], F32, kind="ExternalInput")
    t_b1 = nc.dram_tensor("b1", [HID_CH, 1], F32, kind="ExternalInput")
    t_w2 = nc.dram_tensor("w2", [HID_CH, OUT_CH], F32, kind="ExternalInput")
    t_b2 = nc.dram_tensor("b2", [OUT_CH, 1], F32, kind="ExternalInput")
    t_out = nc.dram_tensor("out", [NSH, OUT_CH], F32, kind="ExternalOutput")

    x2_shard = nc.dram_tensor("x2_shard", [P, NWRAP], BF16)
    x2_full = nc.dram_tensor("x2_full", [N_CORES * P, NWRAP], BF16,
                             addr_space="Shared" if SHARED_AG else "Local")

    rg = [list(range(N_CORES))]
    gq = [0]  # round-robin gather queue counter

    with tile.TileContext(nc) as tc:
        with (
            tc.tile_pool(name="const", bufs=1) as cp,
            tc.tile_pool(name="sbuf", bufs=3) as sb,
            tc.tile_pool(name="gpool", bufs=2) as gp,
            tc.tile_pool(name="opool", bufs=3) as op,
            tc.tile_pool(name="psum", bufs=2, space="PSUM") as ps,
        ):
            nc.gpsimd.load_library(mlp)

            idx_sb = cp.tile([P, T_total * 8], mybir.dt.int16)
            nc.sync.dma_start(out=idx_sb[:], in_=t_idxw[:, :])
            dstrel_sb = cp.tile([P, T_total], F32)
            nc.sync.dma_start(out=dstrel_sb[:], in_=t_dstrel[:, :])
            disw_sb = cp.tile([P, NBLK], F32)
            nc.sync.dma_start(out=disw_sb[:], in_=t_disw[:, :])
            diswT_sb = cp.tile([P, NWRAP], F32)
            nc.sync.dma_start(out=diswT_sb[:], in_=t_diswT[:, :])

            iota_i = cp.tile([P, P], mybir.dt.int32)
            nc.gpsimd.iota(iota_i[:], pattern=[[1, P]], base=0, channel_multiplier=0)
            iota_bf = cp.tile([P, P], BF16)
            nc.vector.tensor_copy(out=iota_bf[:], in_=iota_i[:])
            iota_rep = cp.tile([P, TMAXRUN, P], BF16)
            for j in range(TMAXRUN):
                nc.vector.tensor_copy(out=iota_rep[:, j, :], in_=iota_bf[:])

            ident_bf = cp.tile([P, P], BF16)
            make_identity(nc, ident_bf[:])
            ident_f = cp.tile([OUT_CH, OUT_CH], F32)
            make_identity(nc, ident_f[:])

            w1_f = cp.tile([IN_CH, HID_CH], F32)
            nc.sync.dma_start(out=w1_f[:], in_=t_w1[:, :])
            w1_bf = cp.tile([IN_CH, HID_CH], BF16)
            nc.vector.tensor_copy(out=w1_bf[:], in_=w1_f[:])
            w2_f = cp.tile([HID_CH, OUT_CH], F32)
            nc.sync.dma_start(out=w2_f[:], in_=t_w2[:, :])
            w2_bf = cp.tile([HID_CH, OUT_CH], BF16)
            nc.vector.tensor_copy(out=w2_bf[:], in_=w2_f[:])
            b1_sb = cp.tile([HID_CH, 1], F32)
            nc.sync.dma_start(out=b1_sb[:], in_=t_b1[:, :])
            b2_sb = cp.tile([OUT_CH, 1], F32)
            nc.sync.dma_start(out=b2_sb[:], in_=t_b2[:, :])

            stage1 = cp.tile([P, NBLK, IN_CH], BF16)
            nc.sync.dma_start(
                out=stage1[:].rearrange("p b c -> p (b c)"), in_=t_x1own[:, :])
            x2stage = cp.tile([P, NBLK, HID_CH], BF16)

            def layer(table, stage, w_bf, b_sb, oc, epilogue):
                tabv = table.ap().rearrange("q (r c) -> (q r) c", c=HID_CH)
                Tg = 0
                for blocks in CHUNKS:
                    tl = [int(t_bh[b, 0]) for b in blocks]
                    th = [int(t_bh[b, 1]) for b in blocks]
                    n_lo = sum(tl)
                    n_hi = sum(th)
                    Tc = n_lo + n_hi
                    g = gp.tile([P, Tc, HID_CH], BF16, tag="g")
                    off = 0
                    for win, (w0, w1r) in enumerate(((0, VLO), (VLO, VTAB))):
                        nrun = (n_lo, n_hi)[win]
                        for n in _chunks(nrun):
                            nc.gpsimd.dma_gather(
                                out_ap=g[:, off:off + n, :], in_ap=tabv[w0:w1r, :],
                                idxs_ap=idx_sb[:, 8 * (Tg + off): 8 * (Tg + off + n)],
                                num_idxs=n * P, num_idxs_reg=n * P,
                                elem_size=HID_CH, queue_num=gq[0] % 4,
                            )
                            gq[0] += 1
                            off += n

                    lo_off = 0
                    hi_off = n_lo
                    for bi, b in enumerate(blocks):
                        tb = tl[bi] + th[bi]
                        ob = op.tile([P, tb, P], BF16, tag="ob")
                        gidx = (list(range(lo_off, lo_off + tl[bi]))
                                + list(range(hi_off, hi_off + th[bi])))
                        # per-tile one-hot via tensor_scalar (per-partition
                        # scalar comparator -> DVE 4x packed mode)
                        for jj, j in enumerate(gidx):
                            nc.vector.tensor_scalar(
                                out=ob[:, jj, :], in0=iota_bf[:],
                                scalar1=dstrel_sb[:, Tg + j: Tg + j + 1],
                                scalar2=None,
                                op0=mybir.AluOpType.is_equal,
                            )
                        tps = ps.tile([HID_CH, P], F32, tag="tps")
                        for jj, j in enumerate(gidx):
                            nc.tensor.matmul(
                                out=tps[:], lhsT=g[:, j, :], rhs=ob[:, jj, :],
                                start=(jj == 0), stop=False,
                            )
                        nc.tensor.matmul(
                            out=tps[:], lhsT=stage[:, b, :], rhs=ident_bf[:],
                            start=(tb == 0), stop=True,
                        )
                        # dst-side Dis as a column scale during PSUM -> SBUF
                        t_sb = sb.tile([HID_CH, P], BF16, tag="tsb")
                        nc.vector.tensor_tensor(
                            out=t_sb[:], in0=tps[:],
                            in1=diswT_sb[:, b * P:(b + 1) * P],
                            op=mybir.AluOpType.mult,
                        )
                        ups = ps.tile([oc, P], F32, tag="ups")
                        nc.tensor.matmul(out=ups[:], lhsT=w_bf[:], rhs=t_sb[:],
                                         start=True, stop=True)
                        epilogue(b, ups)
                        lo_off += tl[bi]
                        hi_off += th[bi]
                    Tg += Tc

            # ---- layer 1 ----
            def epi1(b, ups):
                h1t = sb.tile([HID_CH, P], BF16, tag="h1t")
                nc.scalar.activation(out=h1t[:], in_=ups[:],
                                     func=mybir.ActivationFunctionType.Relu,
                                     bias=b1_sb[:, :1])
                trp = ps.tile([P, HID_CH], BF16, tag="trp")
                nc.tensor.transpose(out=trp[:], in_=h1t[:], identity=ident_bf[:])
                nc.vector.tensor_tensor(
                    out=x2stage[:, b, :], in0=trp[:],
                    in1=disw_sb[:, b:b + 1].to_broadcast([P, HID_CH]),
                    op=mybir.AluOpType.mult,
                )

            layer(t_x1tab, stage1, w1_bf, b1_sb, HID_CH, epi1)

            nc.sync.dma_start(
                out=x2_shard[:, :], in_=x2stage[:].rearrange("p b c -> p (b c)"))
            nc.gpsimd.collective_compute(
                "AllGather", mybir.AluOpType.bypass, replica_groups=rg,
                ins=[x2_shard.ap().opt()], outs=[x2_full.ap().opt()],
            )

            # ---- layer 2 ----
            def epi2(b, ups):
                nb = P if b < NFULL else NTAIL
                h2t = sb.tile([OUT_CH, P], F32, tag="h2t")
                nc.scalar.activation(out=h2t[:], in_=ups[:],
                                     func=mybir.ActivationFunctionType.Relu,
                                     bias=b2_sb[:, :1])
                trp2 = ps.tile([P, OUT_CH], F32, tag="trp2")
                nc.tensor.transpose(out=trp2[:], in_=h2t[:], identity=ident_f[:])
                outt = sb.tile([P, OUT_CH], F32, tag="outt")
                nc.vector.tensor_copy(out=outt[:], in_=trp2[:])
                nc.sync.dma_start(out=t_out[b * P: b * P + nb, :], in_=outt[:nb, :])

            layer(x2_full, x2stage, w2_bf, b2_sb, OUT_CH, epi2)

    nc.compile()
    return nc


def kernel(x, edge_index, W1, b1, W2, b2, _trace=False):
    global LAST_RESULT
    x = np.asarray(x, dtype=np.float32)
    edge_index = np.asarray(edge_index, dtype=np.int32)

    prep = _host_prep(x, edge_index)
    nc = _build(prep)
    in_maps = _make_in_maps(prep, W1, b1, W2, b2)

    res = run_bass_kernel_spmd(nc, in_maps, core_ids=list(range(N_CORES)),
                               trace=_trace)
    LAST_RESULT = res
    out = np.concatenate([res.results[k]["out"] for k in range(N_CORES)], axis=0)
    return out.astype(np.float32)


# revision 13
# speedup vs baseline: 1.5121x; 1.0111x over previous
"""2-layer GCN (GCNConv 128->128->64, N=50000, E=800000) on 8 TRN2 NeuronCores.

Strategy v3 (dst-sharded, aggregate-first, host-built layer-1 table,
piece-pipelined AllGather):
  out = relu(A_hat @ (relu(A_hat @ x @ W1 + b1)) @ W2 + b2),  A_hat = D^-1/2 (A+I) D^-1/2
  - Layer-1 gather table x1' = Dis*x is built ON HOST (bf16, wrapped layout) and
    uploaded in full to every core: no first AllGather.
  - Dst-side Dis is applied once per 128-node block as a column scale on the
    aggregated PSUM tile; src-side Dis is folded into the tables. One-hot
    scatter matrices are iota==dst_rel built with tensor_scalar (DVE 4x mode).
  - The layer-2 table (Dis*relu(h1), bf16) is exchanged in NCC piece-contiguous
    AllGathers, each fired as soon as its block range finishes layer 1 — the
    exchange overlaps the tail of layer 1. The x2 table is laid out
    piece-major so every collective touches one contiguous region.
  - Edges are grouped (dst-core, chunk of CH dst blocks, int16 window, block),
    tiled in 128-edge tiles, gathered 8 tiles per dma_gather call (the ucode
    SWDGE ring caps one call at 1024 descriptors), round-robin over 4 queues.
Host-side work is index prep + the x1' table build; output concat at the end.
"""

import numpy as np
import ml_dtypes

import concourse.bass as bass
import concourse.bacc as bacc
import concourse.mybir as mybir
import concourse.tile as tile
from concourse.bass_utils import run_bass_kernel_spmd
from concourse.library_config import mlp
from concourse.masks import make_identity

P = 128
N_NODES = 50000
N_EDGES = 800000
IN_CH = 128
HID_CH = 128
OUT_CH = 64
N_CORES = 8
NSH = N_NODES // N_CORES           # 6250 nodes per core
NBLK = (NSH + P - 1) // P          # 49 blocks per core (48 full + 106)
NFULL = NSH // P                   # 48
NTAIL = NSH - NFULL * P            # 106
NWRAP = NBLK * P                   # 6272
VLO = 32768                        # low table half (int16 index range)
VTAB = N_CORES * P * NBLK          # 50176 table rows in [*, 128] view
CH = 4                             # dst blocks per gather chunk
CHUNKS = [list(range(c, min(c + CH, NBLK))) for c in range(0, NBLK, CH)]
import os as _os
# ucode's SWDGE ring is fixed at 1024 descriptors per call regardless of the
# scratch size; >1024-idx gather calls crash NRT.
GCAP = int(_os.environ.get("K_GCAP", "8"))     # tiles (x128 idxs) per gather call
SHARED_AG = _os.environ.get("K_SHARED", "1") == "1"   # Shared-output AllGather
NCC = int(_os.environ.get("K_NCC", "3"))       # AllGather pieces (overlap w/ L1)

BF16 = mybir.dt.bfloat16
F32 = mybir.dt.float32
NPBF16 = ml_dtypes.bfloat16

LAST_RESULT = None  # for test harness: BassKernelResults of last run


def _chunks(t, cap=GCAP):
    """Split t tiles into balanced chunks of <= cap (16 -> 8+8, not 15+1)."""
    if t == 0:
        return []
    n = -(-t // cap)
    base, rem = divmod(t, n)
    return [base + (1 if i < rem else 0) for i in range(n)]


def _pieces():
    """AllGather piece plan: list of (b0, b1, end_chunk_idx)."""
    bounds = [round(i * len(CHUNKS) / NCC) for i in range(NCC + 1)]
    out = []
    for p in range(NCC):
        c0, c1 = bounds[p], bounds[p + 1]
        if c0 == c1:
            continue
        out.append((CHUNKS[c0][0], CHUNKS[c1 - 1][-1] + 1, c1 - 1))
    return out


def _edge_plan(srow, dst):
    """Slot plan for one layer given each edge's gather-table row `srow`.

    Returns dict with t_bh [NBLK,2], T_total, idxw [C,128,T*8] i16,
    dstrelw [C,128,T] f32.
    """
    half = (srow >= VLO).astype(np.int64)
    core_of = dst // NSH
    io = dst - core_of * NSH
    blk = io // P
    drel = io % P

    cnt = np.bincount((core_of * NBLK + blk) * 2 + half,
                      minlength=N_CORES * NBLK * 2).reshape(N_CORES, NBLK, 2)
    t_bh = np.ceil(cnt.max(axis=0) / P).astype(np.int64)   # [NBLK, 2]

    sect_order = []
    for blocks in CHUNKS:
        for b in blocks:
            sect_order.append((b, 0))
        for b in blocks:
            sect_order.append((b, 1))
    sect_tiles = np.array([t_bh[b, h] for b, h in sect_order], np.int64)
    sect_tile_base = np.concatenate([[0], np.cumsum(sect_tiles)])[:-1]
    T_total = int(sect_tiles.sum())
    EPC = T_total * P

    NSECT = len(sect_order)
    sect_index = np.zeros((NBLK, 2), np.int64)
    for s, (b, h) in enumerate(sect_order):
        sect_index[b, h] = s

    gkey = core_of * NSECT + sect_index[blk, half]
    order = np.argsort(gkey, kind="stable")
    g_sorted = gkey[order]
    gcnt = np.bincount(g_sorted, minlength=N_CORES * NSECT)
    group_start = np.concatenate([[0], np.cumsum(gcnt)])[:-1]
    pos_in_group = np.arange(len(srow)) - group_start[g_sorted]
    slot = sect_tile_base[g_sorted % NSECT] * P + pos_in_group
    edge_core = g_sorted // NSECT

    idx_rows = np.zeros((N_CORES, EPC), np.int64)
    dstrel = np.full((N_CORES, EPC), -1.0, np.float32)
    idx_rows[edge_core, slot] = (srow - half * VLO)[order]
    dstrel[edge_core, slot] = drel[order].astype(np.float32)

    idxw = idx_rows.reshape(N_CORES, EPC // 16, 16).transpose(0, 2, 1)
    idxw = np.ascontiguousarray(np.tile(idxw, (1, 8, 1))).astype(np.int16)
    dstrelw = np.ascontiguousarray(
        dstrel.reshape(N_CORES, T_total, P).transpose(0, 2, 1)).astype(np.float32)
    return {"t_bh": t_bh, "T_total": T_total, "idxw": idxw, "dstrelw": dstrelw}


def _host_prep(x, edge_index):
    """Index prep + host-built layer-1 gather table (bf16, wrapped layout)."""
    src = edge_index[0].astype(np.int64)
    dst = edge_index[1].astype(np.int64)
    deg = np.bincount(dst, minlength=N_NODES) + 1   # + self loop
    dis = (1.0 / np.sqrt(deg.astype(np.float64))).astype(np.float32)

    # layer-1 table: row q = k*128 + (i%128), block col r = i//128
    x1 = x * dis[:, None]
    v = np.arange(N_NODES)
    kk = v // NSH
    ii = v % NSH
    q = kk * P + (ii % P)
    r = ii // P
    x1tab = np.zeros((N_CORES * P, NBLK, IN_CH), np.float32)
    x1tab[q, r] = x1
    x1tab = np.ascontiguousarray(x1tab.reshape(N_CORES * P, NWRAP)).astype(NPBF16)

    sk = src // NSH
    si = src - sk * NSH
    sq = sk * P + (si % P)
    sr = si // P
    srow1 = sq * NBLK + sr

    # layer-2 table is piece-major: piece p holds blocks [b0, b1) as a
    # contiguous [1024, (b1-b0)*128] region
    pieces = _pieces()
    blk_piece_base = np.zeros(NBLK, np.int64)   # row base of (block, q=0)
    blk_piece_nb = np.zeros(NBLK, np.int64)     # piece width in blocks
    blk_piece_b0 = np.zeros(NBLK, np.int64)
    rowbase = 0
    for (b0, b1, _e) in pieces:
        nb = b1 - b0
        blk_piece_base[b0:b1] = rowbase
        blk_piece_nb[b0:b1] = nb
        blk_piece_b0[b0:b1] = b0
        rowbase += N_CORES * P * nb
    srow2 = (blk_piece_base[sr] + sq * blk_piece_nb[sr]
             + (sr - blk_piece_b0[sr]))

    plan1 = _edge_plan(srow1, dst)
    plan2 = _edge_plan(srow2, dst)

    disw = np.zeros((N_CORES, P, NBLK), np.float32)
    nodes = np.arange(NBLK * P)
    valid = nodes < NSH
    for c in range(N_CORES):
        vv = np.zeros(NBLK * P, np.float32)
        vv[valid] = dis[c * NSH + nodes[valid]]
        disw[c] = vv.reshape(NBLK, P).T
    diswT = np.ascontiguousarray(np.broadcast_to(
        disw.transpose(0, 2, 1).reshape(N_CORES, 1, NWRAP), (N_CORES, P, NWRAP)))

    return {
        "x1tab": x1tab,
        "x1own": [np.ascontiguousarray(x1tab[c * P:(c + 1) * P])
                  for c in range(N_CORES)],
        "plan1": plan1, "plan2": plan2,
        "disw": disw, "diswT": diswT,
    }


def _make_in_maps(prep, W1, b1, W2, b2):
    maps = []
    for c in range(N_CORES):
        maps.append({
            "x1tab": prep["x1tab"],
            "x1own": prep["x1own"][c],
            "idxw1": np.ascontiguousarray(prep["plan1"]["idxw"][c]),
            "dstrelw1": np.ascontiguousarray(prep["plan1"]["dstrelw"][c]),
            "idxw2": np.ascontiguousarray(prep["plan2"]["idxw"][c]),
            "dstrelw2": np.ascontiguousarray(prep["plan2"]["dstrelw"][c]),
            "disw": np.ascontiguousarray(prep["disw"][c]),
            "diswT": np.ascontiguousarray(prep["diswT"][c]),
            "w1": np.asarray(W1, np.float32),
            "b1": np.ascontiguousarray(np.asarray(b1, np.float32).reshape(HID_CH, 1)),
            "w2": np.asarray(W2, np.float32),
            "b2": np.ascontiguousarray(np.asarray(b2, np.float32).reshape(OUT_CH, 1)),
        })
    return maps


def _build(prep):
    plan1, plan2 = prep["plan1"], prep["plan2"]
    T1, T2 = plan1["T_total"], plan2["T_total"]
    TMAXRUN = int(max(plan1["t_bh"].max(), plan2["t_bh"].max()))
    pieces = _pieces()

    nc = bacc.Bacc("TRN2", target_bir_lowering=False, num_devices=N_CORES,
                   num_swdge_queues=4)

    t_x1tab = nc.dram_tensor("x1tab", [N_CORES * P, NWRAP], BF16, kind="ExternalInput")
    t_x1own = nc.dram_tensor("x1own", [P, NWRAP], BF16, kind="ExternalInput")
    t_idxw1 = nc.dram_tensor("idxw1", [P, T1 * 8], mybir.dt.int16, kind="ExternalInput")
    t_dstrel1 = nc.dram_tensor("dstrelw1", [P, T1], F32, kind="ExternalInput")
    t_idxw2 = nc.dram_tensor("idxw2", [P, T2 * 8], mybir.dt.int16, kind="ExternalInput")
    t_dstrel2 = nc.dram_tensor("dstrelw2", [P, T2], F32, kind="ExternalInput")
    t_disw = nc.dram_tensor("disw", [P, NBLK], F32, kind="ExternalInput")
    t_diswT = nc.dram_tensor("diswT", [P, NWRAP], F32, kind="ExternalInput")
    t_w1 = nc.dram_tensor("w1", [IN_CH, HID_CH], F32, kind="ExternalInput")
    t_b1 = nc.dram_tensor("b1", [HID_CH, 1], F32, kind="ExternalInput")
    t_w2 = nc.dram_tensor("w2", [HID_CH, OUT_CH], F32, kind="ExternalInput")
    t_b2 = nc.dram_tensor("b2", [OUT_CH, 1], F32, kind="ExternalInput")
    t_out = nc.dram_tensor("out", [NSH, OUT_CH], F32, kind="ExternalOutput")

    # piece-major flat buffers so each AllGather is contiguous
    x2_shard = nc.dram_tensor("x2_shard", [1, P * NWRAP], BF16)
    x2_full = nc.dram_tensor("x2_full", [1, N_CORES * P * NWRAP], BF16,
                             addr_space="Shared" if SHARED_AG else "Local")

    rg = [list(range(N_CORES))]
    gq = [0]  # round-robin gather queue counter

    with tile.TileContext(nc) as tc:
        with (
            tc.tile_pool(name="const", bufs=1) as cp,
            tc.tile_pool(name="sbuf", bufs=3) as sb,
            tc.tile_pool(name="gpool", bufs=2) as gp,
            tc.tile_pool(name="opool", bufs=3) as op,
            tc.tile_pool(name="psum", bufs=2, space="PSUM") as ps,
        ):
            nc.gpsimd.load_library(mlp)

            idx_sb1 = cp.tile([P, T1 * 8], mybir.dt.int16)
            nc.sync.dma_start(out=idx_sb1[:], in_=t_idxw1[:, :])
            dstrel_sb1 = cp.tile([P, T1], F32)
            nc.sync.dma_start(out=dstrel_sb1[:], in_=t_dstrel1[:, :])
            idx_sb2 = cp.tile([P, T2 * 8], mybir.dt.int16)
            nc.sync.dma_start(out=idx_sb2[:], in_=t_idxw2[:, :])
            dstrel_sb2 = cp.tile([P, T2], F32)
            nc.sync.dma_start(out=dstrel_sb2[:], in_=t_dstrel2[:, :])
            disw_sb = cp.tile([P, NBLK], F32)
            nc.sync.dma_start(out=disw_sb[:], in_=t_disw[:, :])
            diswT_sb = cp.tile([P, NWRAP], F32)
            nc.sync.dma_start(out=diswT_sb[:], in_=t_diswT[:, :])

            iota_i = cp.tile([P, P], mybir.dt.int32)
            nc.gpsimd.iota(iota_i[:], pattern=[[1, P]], base=0, channel_multiplier=0)
            iota_bf = cp.tile([P, P], BF16)
            nc.vector.tensor_copy(out=iota_bf[:], in_=iota_i[:])

            ident_bf = cp.tile([P, P], BF16)
            make_identity(nc, ident_bf[:])
            ident_f = cp.tile([OUT_CH, OUT_CH], F32)
            make_identity(nc, ident_f[:])

            w1_f = cp.tile([IN_CH, HID_CH], F32)
            nc.sync.dma_start(out=w1_f[:], in_=t_w1[:, :])
            w1_bf = cp.tile([IN_CH, HID_CH], BF16)
            nc.vector.tensor_copy(out=w1_bf[:], in_=w1_f[:])
            w2_f = cp.tile([HID_CH, OUT_CH], F32)
            nc.sync.dma_start(out=w2_f[:], in_=t_w2[:, :])
            w2_bf = cp.tile([HID_CH, OUT_CH], BF16)
            nc.vector.tensor_copy(out=w2_bf[:], in_=w2_f[:])
            b1_sb = cp.tile([HID_CH, 1], F32)
            nc.sync.dma_start(out=b1_sb[:], in_=t_b1[:, :])
            b2_sb = cp.tile([OUT_CH, 1], F32)
            nc.sync.dma_start(out=b2_sb[:], in_=t_b2[:, :])

            stage1 = cp.tile([P, NBLK, IN_CH], BF16)
            nc.sync.dma_start(
                out=stage1[:].rearrange("p b c -> p (b c)"), in_=t_x1own[:, :])
            x2stage = cp.tile([P, NBLK, HID_CH], BF16)

            def layer(tabv, plan, idx_sb, dstrel_sb, stage, w_bf, b_sb, oc,
                      epilogue, after_chunk=None):
                t_bh = plan["t_bh"]
                Tg = 0
                for ci, blocks in enumerate(CHUNKS):
                    tl = [int(t_bh[b, 0]) for b in blocks]
                    th = [int(t_bh[b, 1]) for b in blocks]
                    n_lo = sum(tl)
                    n_hi = sum(th)
                    Tc = n_lo + n_hi
                    g = gp.tile([P, Tc, HID_CH], BF16, tag="g")
                    off = 0
                    for win, (w0, w1r) in enumerate(((0, VLO), (VLO, VTAB))):
                        nrun = (n_lo, n_hi)[win]
                        for n in _chunks(nrun):
                            nc.gpsimd.dma_gather(
                                out_ap=g[:, off:off + n, :], in_ap=tabv[w0:w1r, :],
                                idxs_ap=idx_sb[:, 8 * (Tg + off): 8 * (Tg + off + n)],
                                num_idxs=n * P, num_idxs_reg=n * P,
                                elem_size=HID_CH, queue_num=gq[0] % 4,
                            )
                            gq[0] += 1
                            off += n

                    lo_off = 0
                    hi_off = n_lo
                    for bi, b in enumerate(blocks):
                        tb = tl[bi] + th[bi]
                        ob = op.tile([P, tb, P], BF16, tag="ob")
                        gidx = (list(range(lo_off, lo_off + tl[bi]))
                                + list(range(hi_off, hi_off + th[bi])))
                        # one-hots: per-partition scalar compare (DVE 4x mode)
                        for jj, j in enumerate(gidx):
                            nc.vector.tensor_scalar(
                                out=ob[:, jj, :], in0=iota_bf[:],
                                scalar1=dstrel_sb[:, Tg + j: Tg + j + 1],
                                scalar2=None,
                                op0=mybir.AluOpType.is_equal,
                            )
                        tps = ps.tile([HID_CH, P], F32, tag="tps")
                        for jj, j in enumerate(gidx):
                            nc.tensor.matmul(
                                out=tps[:], lhsT=g[:, j, :], rhs=ob[:, jj, :],
                                start=(jj == 0), stop=False,
                            )
                        nc.tensor.matmul(
                            out=tps[:], lhsT=stage[:, b, :], rhs=ident_bf[:],
                            start=(tb == 0), stop=True,
                        )
                        # dst-side Dis as a column scale during PSUM -> SBUF
                        t_sb = sb.tile([HID_CH, P], BF16, tag="tsb")
                        nc.vector.tensor_tensor(
                            out=t_sb[:], in0=tps[:],
                            in1=diswT_sb[:, b * P:(b + 1) * P],
                            op=mybir.AluOpType.mult,
                        )
                        ups = ps.tile([oc, P], F32, tag="ups")
                        nc.tensor.matmul(out=ups[:], lhsT=w_bf[:], rhs=t_sb[:],
                                         start=True, stop=True)
                        epilogue(b, ups)
                        lo_off += tl[bi]
                        hi_off += th[bi]
                    Tg += Tc
                    if after_chunk is not None:
                        after_chunk(ci)

            # ---- layer 1 (+ pipelined AllGather pieces) ----
            def epi1(b, ups):
                h1t = sb.tile([HID_CH, P], BF16, tag="h1t")
                nc.scalar.activation(out=h1t[:], in_=ups[:],
                                     func=mybir.ActivationFunctionType.Relu,
                                     bias=b1_sb[:, :1])
                trp = ps.tile([P, HID_CH], BF16, tag="trp")
                nc.tensor.transpose(out=trp[:], in_=h1t[:], identity=ident_bf[:])
                nc.vector.tensor_tensor(
                    out=x2stage[:, b, :], in0=trp[:],
                    in1=disw_sb[:, b:b + 1].to_broadcast([P, HID_CH]),
                    op=mybir.AluOpType.mult,
                )

            piece_at_chunk = {e: (b0, b1) for (b0, b1, e) in pieces}
            piece_elem_base = {}
            acc = 0
            for (b0, b1, e) in pieces:
                piece_elem_base[e] = acc
                acc += (b1 - b0) * HID_CH

            def ag_piece(ci):
                if ci not in piece_at_chunk:
                    return
                b0, b1 = piece_at_chunk[ci]
                span = (b1 - b0) * HID_CH
                base = piece_elem_base[ci]
                shv = x2_shard.ap()[0:1, P * base: P * (base + span)] \
                    .rearrange("o (p s) -> (o p) s", s=span)
                fuv = x2_full.ap()[0:1, N_CORES * P * base:
                                   N_CORES * P * (base + span)] \
                    .rearrange("o (q s) -> (o q) s", s=span)
                nc.sync.dma_start(
                    out=shv,
                    in_=x2stage[:, b0:b1, :].rearrange("p b c -> p (b c)"))
                nc.gpsimd.collective_compute(
                    "AllGather", mybir.AluOpType.bypass, replica_groups=rg,
                    ins=[shv.opt()], outs=[fuv.opt()],
                )

            tabv1 = t_x1tab.ap().rearrange("q (r c) -> (q r) c", c=HID_CH)
            layer(tabv1, plan1, idx_sb1, dstrel_sb1, stage1, w1_bf, b1_sb,
                  HID_CH, epi1, after_chunk=ag_piece)

            # ---- layer 2 ----
            def epi2(b, ups):
                nb = P if b < NFULL else NTAIL
                h2t = sb.tile([OUT_CH, P], F32, tag="h2t")
                nc.scalar.activation(out=h2t[:], in_=ups[:],
                                     func=mybir.ActivationFunctionType.Relu,
                                     bias=b2_sb[:, :1])
                trp2 = ps.tile([P, OUT_CH], F32, tag="trp2")
                nc.tensor.transpose(out=trp2[:], in_=h2t[:], identity=ident_f[:])
                outt = sb.tile([P, OUT_CH], F32, tag="outt")
                nc.vector.tensor_copy(out=outt[:], in_=trp2[:])
                nc.sync.dma_start(out=t_out[b * P: b * P + nb, :], in_=outt[:nb, :])

            tabv2 = x2_full.ap().rearrange("o (v c) -> (o v) c", c=HID_CH)
            layer(tabv2, plan2, idx_sb2, dstrel_sb2, x2stage, w2_bf, b2_sb,
                  OUT_CH, epi2)

    nc.compile()
    return nc


def kernel(x, edge_index, W1, b1, W2, b2, _trace=False):
    global LAST_RESULT
    x = np.asarray(x, dtype=np.float32)
    edge_index = np.asarray(edge_index, dtype=np.int32)

    prep = _host_prep(x, edge_index)
    nc = _build(prep)
    in_maps = _make_in_maps(prep, W1, b1, W2, b2)

    res = run_bass_kernel_spmd(nc, in_maps, core_ids=list(range(N_CORES)),
                               trace=_trace)
    LAST_RESULT = res
    out = np.concatenate([res.results[k]["out"] for k in range(N_CORES)], axis=0)
    return out.astype(np.float32)


# revision 16
# speedup vs baseline: 2.2378x; 1.4800x over previous
"""2-layer GCN (GCNConv 128->128->64, N=50000, E=800000) on 8 TRN2 NeuronCores.

Strategy v3 (dst-sharded, aggregate-first, host-built layer-1 table,
piece-pipelined AllGather):
  out = relu(A_hat @ (relu(A_hat @ x @ W1 + b1)) @ W2 + b2),  A_hat = D^-1/2 (A+I) D^-1/2
  - Layer-1 gather table x1' = Dis*x is built ON HOST (bf16, wrapped layout) and
    uploaded in full to every core: no first AllGather.
  - Dst-side Dis is applied once per 128-node block as a column scale on the
    aggregated PSUM tile; src-side Dis is folded into the tables. One-hot
    scatter matrices are iota==dst_rel built with tensor_scalar (DVE 4x mode).
  - The layer-2 table (Dis*relu(h1), bf16) is exchanged in NCC piece-contiguous
    AllGathers, each fired as soon as its block range finishes layer 1 — the
    exchange overlaps the tail of layer 1. The x2 table is laid out
    piece-major so every collective touches one contiguous region.
  - Edges are grouped (dst-core, chunk of CH dst blocks, int16 window, block),
    tiled in 128-edge tiles, gathered 8 tiles per dma_gather call (the ucode
    SWDGE ring caps one call at 1024 descriptors), round-robin over 4 queues.
Host-side work is index prep + the x1' table build; output concat at the end.
"""

import numpy as np
import ml_dtypes

import concourse.bass as bass
import concourse.bacc as bacc
import concourse.mybir as mybir
import concourse.tile as tile
from concourse.bass_utils import run_bass_kernel_spmd
from concourse.library_config import mlp
from concourse.masks import make_identity

P = 128
N_NODES = 50000
N_EDGES = 800000
IN_CH = 128
HID_CH = 128
OUT_CH = 64
N_CORES = 8
NSH = N_NODES // N_CORES           # 6250 nodes per core
NBLK = (NSH + P - 1) // P          # 49 blocks per core (48 full + 106)
NFULL = NSH // P                   # 48
NTAIL = NSH - NFULL * P            # 106
NWRAP = NBLK * P                   # 6272
VLO = 32768                        # low table half (int16 index range)
VTAB = N_CORES * P * NBLK          # 50176 table rows in [*, 128] view
CH = 4                             # dst blocks per gather chunk
CHUNKS = [list(range(c, min(c + CH, NBLK))) for c in range(0, NBLK, CH)]
import os as _os
# ucode's SWDGE ring is fixed at 1024 descriptors per call regardless of the
# scratch size; >1024-idx gather calls crash NRT.
GCAP = int(_os.environ.get("K_GCAP", "8"))     # tiles (x128 idxs) per gather call
SHARED_AG = _os.environ.get("K_SHARED", "1") == "1"   # Shared-output AllGather
NCC = int(_os.environ.get("K_NCC", "3"))       # AllGather pieces (overlap w/ L1)

BF16 = mybir.dt.bfloat16
F32 = mybir.dt.float32
NPBF16 = ml_dtypes.bfloat16

LAST_RESULT = None  # for test harness: BassKernelResults of last run


def _chunks(t, cap=GCAP):
    """Split t tiles into balanced chunks of <= cap (16 -> 8+8, not 15+1)."""
    if t == 0:
        return []
    n = -(-t // cap)
    base, rem = divmod(t, n)
    return [base + (1 if i < rem else 0) for i in range(n)]


def _pieces():
    """AllGather piece plan: list of (b0, b1, end_chunk_idx)."""
    bounds = [round(i * len(CHUNKS) / NCC) for i in range(NCC + 1)]
    out = []
    for p in range(NCC):
        c0, c1 = bounds[p], bounds[p + 1]
        if c0 == c1:
            continue
        out.append((CHUNKS[c0][0], CHUNKS[c1 - 1][-1] + 1, c1 - 1))
    return out


def _edge_plan(srow, dst):
    """Slot plan for one layer given each edge's gather-table row `srow`.

    Returns dict with t_bh [NBLK,2], T_total, idxw [C,128,T*8] i16,
    dstrelw [C,128,T] f32.
    """
    half = (srow >= VLO).astype(np.int64)
    core_of = dst // NSH
    io = dst - core_of * NSH
    blk = io // P
    drel = io % P

    cnt = np.bincount((core_of * NBLK + blk) * 2 + half,
                      minlength=N_CORES * NBLK * 2).reshape(N_CORES, NBLK, 2)
    t_bh = np.ceil(cnt.max(axis=0) / P).astype(np.int64)   # [NBLK, 2]

    sect_order = []
    for blocks in CHUNKS:
        for b in blocks:
            sect_order.append((b, 0))
        for b in blocks:
            sect_order.append((b, 1))
    sect_tiles = np.array([t_bh[b, h] for b, h in sect_order], np.int64)
    sect_tile_base = np.concatenate([[0], np.cumsum(sect_tiles)])[:-1]
    T_total = int(sect_tiles.sum())
    EPC = T_total * P

    NSECT = len(sect_order)
    sect_index = np.zeros((NBLK, 2), np.int64)
    for s, (b, h) in enumerate(sect_order):
        sect_index[b, h] = s

    gkey = core_of * NSECT + sect_index[blk, half]
    order = np.argsort(gkey, kind="stable")
    g_sorted = gkey[order]
    gcnt = np.bincount(g_sorted, minlength=N_CORES * NSECT)
    group_start = np.concatenate([[0], np.cumsum(gcnt)])[:-1]
    pos_in_group = np.arange(len(srow)) - group_start[g_sorted]
    slot = sect_tile_base[g_sorted % NSECT] * P + pos_in_group
    edge_core = g_sorted // NSECT

    idx_rows = np.zeros((N_CORES, EPC), np.int64)
    dstrel = np.full((N_CORES, EPC), -1.0, np.float32)
    idx_rows[edge_core, slot] = (srow - half * VLO)[order]
    dstrel[edge_core, slot] = drel[order].astype(np.float32)

    idxw = idx_rows.reshape(N_CORES, EPC // 16, 16).transpose(0, 2, 1)
    idxw = np.ascontiguousarray(np.tile(idxw, (1, 8, 1))).astype(np.int16)
    dstrelw = np.ascontiguousarray(
        dstrel.reshape(N_CORES, T_total, P).transpose(0, 2, 1)).astype(np.float32)
    return {"t_bh": t_bh, "T_total": T_total, "idxw": idxw, "dstrelw": dstrelw}


def _host_prep(x, edge_index):
    """Index prep + host-built layer-1 gather table (bf16, wrapped layout)."""
    src = edge_index[0].astype(np.int64)
    dst = edge_index[1].astype(np.int64)
    deg = np.bincount(dst, minlength=N_NODES) + 1   # + self loop
    dis = (1.0 / np.sqrt(deg.astype(np.float64))).astype(np.float32)

    # layer-1 table: row q = k*128 + (i%128), block col r = i//128
    x1 = x * dis[:, None]
    v = np.arange(N_NODES)
    kk = v // NSH
    ii = v % NSH
    q = kk * P + (ii % P)
    r = ii // P
    x1tab = np.zeros((N_CORES * P, NBLK, IN_CH), np.float32)
    x1tab[q, r] = x1
    x1tab = np.ascontiguousarray(x1tab.reshape(N_CORES * P, NWRAP)).astype(NPBF16)

    sk = src // NSH
    si = src - sk * NSH
    sq = sk * P + (si % P)
    sr = si // P
    srow1 = sq * NBLK + sr

    # layer-2 table is piece-major: piece p holds blocks [b0, b1) as a
    # contiguous [1024, (b1-b0)*128] region
    pieces = _pieces()
    blk_piece_base = np.zeros(NBLK, np.int64)   # row base of (block, q=0)
    blk_piece_nb = np.zeros(NBLK, np.int64)     # piece width in blocks
    blk_piece_b0 = np.zeros(NBLK, np.int64)
    rowbase = 0
    for (b0, b1, _e) in pieces:
        nb = b1 - b0
        blk_piece_base[b0:b1] = rowbase
        blk_piece_nb[b0:b1] = nb
        blk_piece_b0[b0:b1] = b0
        rowbase += N_CORES * P * nb
    srow2 = (blk_piece_base[sr] + sq * blk_piece_nb[sr]
             + (sr - blk_piece_b0[sr]))

    plan1 = _edge_plan(srow1, dst)
    plan2 = _edge_plan(srow2, dst)

    disw = np.zeros((N_CORES, P, NBLK), np.float32)
    nodes = np.arange(NBLK * P)
    valid = nodes < NSH
    for c in range(N_CORES):
        vv = np.zeros(NBLK * P, np.float32)
        vv[valid] = dis[c * NSH + nodes[valid]]
        disw[c] = vv.reshape(NBLK, P).T
    diswT = np.ascontiguousarray(np.broadcast_to(
        disw.transpose(0, 2, 1).reshape(N_CORES, 1, NWRAP), (N_CORES, P, NWRAP)))

    return {
        "x1tab": x1tab,
        "x1own": [np.ascontiguousarray(x1tab[c * P:(c + 1) * P])
                  for c in range(N_CORES)],
        "plan1": plan1, "plan2": plan2,
        "disw": disw, "diswT": diswT,
    }


def _pack_layout(T1, T2):
    """Byte layout of the single packed input buffer (512B-aligned segments)."""
    segs = {}
    off = 0

    def add(name, nbytes):
        nonlocal off
        segs[name] = off
        off += (nbytes + 511) // 512 * 512

    add("x1tab", N_CORES * P * NWRAP * 2)
    add("x1own", P * NWRAP * 2)
    add("idxw1", P * T1 * 8 * 2)
    add("idxw2", P * T2 * 8 * 2)
    add("dstrelw1", P * T1 * 4)
    add("dstrelw2", P * T2 * 4)
    add("disw", P * NBLK * 4)
    add("diswT", P * NWRAP * 4)
    add("w1", IN_CH * HID_CH * 4)
    add("b1", HID_CH * 4)
    add("w2", HID_CH * OUT_CH * 4)
    add("b2", OUT_CH * 4)
    segs["_total"] = (off + 511) // 512 * 512
    return segs


def _make_in_maps(prep, W1, b1, W2, b2):
    p1, p2 = prep["plan1"], prep["plan2"]
    segs = _pack_layout(p1["T_total"], p2["T_total"])
    NB = segs["_total"]
    maps = []
    for c in range(N_CORES):
        pk = np.zeros(NB, np.uint8)

        def put(name, arr):
            b = np.ascontiguousarray(arr).view(np.uint8).reshape(-1)
            pk[segs[name]:segs[name] + b.size] = b

        put("x1tab", prep["x1tab"])
        put("x1own", prep["x1own"][c])
        put("idxw1", p1["idxw"][c])
        put("idxw2", p2["idxw"][c])
        put("dstrelw1", p1["dstrelw"][c])
        put("dstrelw2", p2["dstrelw"][c])
        put("disw", prep["disw"][c])
        put("diswT", prep["diswT"][c])
        put("w1", np.asarray(W1, np.float32))
        put("b1", np.asarray(b1, np.float32).reshape(HID_CH, 1))
        put("w2", np.asarray(W2, np.float32))
        put("b2", np.asarray(b2, np.float32).reshape(OUT_CH, 1))
        maps.append({"pack": pk.reshape(1, NB)})
    return maps


def _build(prep):
    plan1, plan2 = prep["plan1"], prep["plan2"]
    T1, T2 = plan1["T_total"], plan2["T_total"]
    TMAXRUN = int(max(plan1["t_bh"].max(), plan2["t_bh"].max()))
    pieces = _pieces()

    nc = bacc.Bacc("TRN2", target_bir_lowering=False, num_devices=N_CORES,
                   num_swdge_queues=4)

    segs = _pack_layout(T1, T2)
    t_pack = nc.dram_tensor("pack", [1, segs["_total"]], mybir.dt.uint8,
                            kind="ExternalInput")
    t_out = nc.dram_tensor("out", [NSH, OUT_CH], F32, kind="ExternalOutput")

    def pview(name, dt, cols):
        """[128, cols] DMA view of a packed segment."""
        nb = P * cols * mybir.dt.size(dt)
        return t_pack.ap()[0:1, segs[name]:segs[name] + nb].bitcast(dt) \
            .rearrange("o (p x) -> (o p) x", x=cols)

    # piece-major flat buffers so each AllGather is contiguous
    x2_shard = nc.dram_tensor("x2_shard", [1, P * NWRAP], BF16)
    x2_full = nc.dram_tensor("x2_full", [1, N_CORES * P * NWRAP], BF16,
                             addr_space="Shared" if SHARED_AG else "Local")

    rg = [list(range(N_CORES))]
    gq = [0]  # round-robin gather queue counter

    with tile.TileContext(nc) as tc:
        with (
            tc.tile_pool(name="const", bufs=1) as cp,
            tc.tile_pool(name="sbuf", bufs=3) as sb,
            tc.tile_pool(name="gpool", bufs=2) as gp,
            tc.tile_pool(name="opool", bufs=3) as op,
            tc.tile_pool(name="psum", bufs=2, space="PSUM") as ps,
        ):
            nc.gpsimd.load_library(mlp)

            idx_sb1 = cp.tile([P, T1 * 8], mybir.dt.int16)
            nc.sync.dma_start(out=idx_sb1[:], in_=pview("idxw1", mybir.dt.int16, T1 * 8))
            dstrel_sb1 = cp.tile([P, T1], F32)
            nc.sync.dma_start(out=dstrel_sb1[:], in_=pview("dstrelw1", F32, T1))
            idx_sb2 = cp.tile([P, T2 * 8], mybir.dt.int16)
            nc.sync.dma_start(out=idx_sb2[:], in_=pview("idxw2", mybir.dt.int16, T2 * 8))
            dstrel_sb2 = cp.tile([P, T2], F32)
            nc.sync.dma_start(out=dstrel_sb2[:], in_=pview("dstrelw2", F32, T2))
            disw_sb = cp.tile([P, NBLK], F32)
            nc.sync.dma_start(out=disw_sb[:], in_=pview("disw", F32, NBLK))
            diswT_sb = cp.tile([P, NWRAP], F32)
            nc.sync.dma_start(out=diswT_sb[:], in_=pview("diswT", F32, NWRAP))

            iota_i = cp.tile([P, P], mybir.dt.int32)
            nc.gpsimd.iota(iota_i[:], pattern=[[1, P]], base=0, channel_multiplier=0)
            iota_bf = cp.tile([P, P], BF16)
            nc.vector.tensor_copy(out=iota_bf[:], in_=iota_i[:])

            ident_bf = cp.tile([P, P], BF16)
            make_identity(nc, ident_bf[:])
            ident_f = cp.tile([OUT_CH, OUT_CH], F32)
            make_identity(nc, ident_f[:])

            w1_f = cp.tile([IN_CH, HID_CH], F32)
            nc.sync.dma_start(out=w1_f[:], in_=pview("w1", F32, HID_CH))
            w1_bf = cp.tile([IN_CH, HID_CH], BF16)
            nc.vector.tensor_copy(out=w1_bf[:], in_=w1_f[:])
            w2_f = cp.tile([HID_CH, OUT_CH], F32)
            nc.sync.dma_start(out=w2_f[:], in_=pview("w2", F32, OUT_CH))
            w2_bf = cp.tile([HID_CH, OUT_CH], BF16)
            nc.vector.tensor_copy(out=w2_bf[:], in_=w2_f[:])
            b1_sb = cp.tile([HID_CH, 1], F32)
            nc.sync.dma_start(out=b1_sb[:], in_=pview("b1", F32, 1))
            b2_sb = cp.tile([OUT_CH, 1], F32)
            nc.sync.dma_start(
                out=b2_sb[:],
                in_=t_pack.ap()[0:1, segs["b2"]:segs["b2"] + OUT_CH * 4]
                .bitcast(F32).rearrange("o (p x) -> (o p) x", x=1))

            stage1 = cp.tile([P, NBLK, IN_CH], BF16)
            nc.sync.dma_start(
                out=stage1[:].rearrange("p b c -> p (b c)"),
                in_=pview("x1own", BF16, NWRAP))
            x2stage = cp.tile([P, NBLK, HID_CH], BF16)

            def layer(tabv, plan, idx_sb, dstrel_sb, stage, w_bf, b_sb, oc,
                      epilogue, after_chunk=None):
                t_bh = plan["t_bh"]
                Tg = 0
                for ci, blocks in enumerate(CHUNKS):
                    tl = [int(t_bh[b, 0]) for b in blocks]
                    th = [int(t_bh[b, 1]) for b in blocks]
                    n_lo = sum(tl)
                    n_hi = sum(th)
                    Tc = n_lo + n_hi
                    g = gp.tile([P, Tc, HID_CH], BF16, tag="g")
                    off = 0
                    for win, (w0, w1r) in enumerate(((0, VLO), (VLO, VTAB))):
                        nrun = (n_lo, n_hi)[win]
                        for n in _chunks(nrun):
                            nc.gpsimd.dma_gather(
                                out_ap=g[:, off:off + n, :], in_ap=tabv[w0:w1r, :],
                                idxs_ap=idx_sb[:, 8 * (Tg + off): 8 * (Tg + off + n)],
                                num_idxs=n * P, num_idxs_reg=n * P,
                                elem_size=HID_CH, queue_num=gq[0] % 4,
                            )
                            gq[0] += 1
                            off += n

                    lo_off = 0
                    hi_off = n_lo
                    for bi, b in enumerate(blocks):
                        tb = tl[bi] + th[bi]
                        ob = op.tile([P, tb, P], BF16, tag="ob")
                        gidx = (list(range(lo_off, lo_off + tl[bi]))
                                + list(range(hi_off, hi_off + th[bi])))
                        # one-hots: per-partition scalar compare (DVE 4x mode)
                        for jj, j in enumerate(gidx):
                            nc.vector.tensor_scalar(
                                out=ob[:, jj, :], in0=iota_bf[:],
                                scalar1=dstrel_sb[:, Tg + j: Tg + j + 1],
                                scalar2=None,
                                op0=mybir.AluOpType.is_equal,
                            )
                        tps = ps.tile([HID_CH, P], F32, tag="tps")
                        for jj, j in enumerate(gidx):
                            nc.tensor.matmul(
                                out=tps[:], lhsT=g[:, j, :], rhs=ob[:, jj, :],
                                start=(jj == 0), stop=False,
                            )
                        nc.tensor.matmul(
                            out=tps[:], lhsT=stage[:, b, :], rhs=ident_bf[:],
                            start=(tb == 0), stop=True,
                        )
                        # dst-side Dis as a column scale during PSUM -> SBUF
                        t_sb = sb.tile([HID_CH, P], BF16, tag="tsb")
                        nc.vector.tensor_tensor(
                            out=t_sb[:], in0=tps[:],
                            in1=diswT_sb[:, b * P:(b + 1) * P],
                            op=mybir.AluOpType.mult,
                        )
                        ups = ps.tile([oc, P], F32, tag="ups")
                        nc.tensor.matmul(out=ups[:], lhsT=w_bf[:], rhs=t_sb[:],
                                         start=True, stop=True)
                        epilogue(b, ups)
                        lo_off += tl[bi]
                        hi_off += th[bi]
                    Tg += Tc
                    if after_chunk is not None:
                        after_chunk(ci)

            # ---- layer 1 (+ pipelined AllGather pieces) ----
            def epi1(b, ups):
                h1t = sb.tile([HID_CH, P], BF16, tag="h1t")
                nc.scalar.activation(out=h1t[:], in_=ups[:],
                                     func=mybir.ActivationFunctionType.Relu,
                                     bias=b1_sb[:, :1])
                trp = ps.tile([P, HID_CH], BF16, tag="trp")
                nc.tensor.transpose(out=trp[:], in_=h1t[:], identity=ident_bf[:])
                nc.vector.tensor_tensor(
                    out=x2stage[:, b, :], in0=trp[:],
                    in1=disw_sb[:, b:b + 1].to_broadcast([P, HID_CH]),
                    op=mybir.AluOpType.mult,
                )

            piece_at_chunk = {e: (b0, b1) for (b0, b1, e) in pieces}
            piece_elem_base = {}
            acc = 0
            for (b0, b1, e) in pieces:
                piece_elem_base[e] = acc
                acc += (b1 - b0) * HID_CH

            def ag_piece(ci):
                if ci not in piece_at_chunk:
                    return
                b0, b1 = piece_at_chunk[ci]
                span = (b1 - b0) * HID_CH
                base = piece_elem_base[ci]
                shv = x2_shard.ap()[0:1, P * base: P * (base + span)] \
                    .rearrange("o (p s) -> (o p) s", s=span)
                fuv = x2_full.ap()[0:1, N_CORES * P * base:
                                   N_CORES * P * (base + span)] \
                    .rearrange("o (q s) -> (o q) s", s=span)
                nc.sync.dma_start(
                    out=shv,
                    in_=x2stage[:, b0:b1, :].rearrange("p b c -> p (b c)"))
                nc.gpsimd.collective_compute(
                    "AllGather", mybir.AluOpType.bypass, replica_groups=rg,
                    ins=[shv.opt()], outs=[fuv.opt()],
                )

            tabv1 = t_pack.ap()[0:1, segs["x1tab"]:
                                segs["x1tab"] + N_CORES * P * NWRAP * 2] \
                .bitcast(BF16).rearrange("o (v c) -> (o v) c", c=HID_CH)
            layer(tabv1, plan1, idx_sb1, dstrel_sb1, stage1, w1_bf, b1_sb,
                  HID_CH, epi1, after_chunk=ag_piece)

            # ---- layer 2 ----
            def epi2(b, ups):
                nb = P if b < NFULL else NTAIL
                h2t = sb.tile([OUT_CH, P], F32, tag="h2t")
                nc.scalar.activation(out=h2t[:], in_=ups[:],
                                     func=mybir.ActivationFunctionType.Relu,
                                     bias=b2_sb[:, :1])
                trp2 = ps.tile([P, OUT_CH], F32, tag="trp2")
                nc.tensor.transpose(out=trp2[:], in_=h2t[:], identity=ident_f[:])
                outt = sb.tile([P, OUT_CH], F32, tag="outt")
                nc.vector.tensor_copy(out=outt[:], in_=trp2[:])
                nc.sync.dma_start(out=t_out[b * P: b * P + nb, :], in_=outt[:nb, :])

            tabv2 = x2_full.ap().rearrange("o (v c) -> (o v) c", c=HID_CH)
            layer(tabv2, plan2, idx_sb2, dstrel_sb2, x2stage, w2_bf, b2_sb,
                  OUT_CH, epi2)

    nc.compile()
    return nc


def kernel(x, edge_index, W1, b1, W2, b2, _trace=False):
    global LAST_RESULT
    x = np.asarray(x, dtype=np.float32)
    edge_index = np.asarray(edge_index, dtype=np.int32)

    prep = _host_prep(x, edge_index)
    nc = _build(prep)
    in_maps = _make_in_maps(prep, W1, b1, W2, b2)

    res = run_bass_kernel_spmd(nc, in_maps, core_ids=list(range(N_CORES)),
                               trace=_trace)
    LAST_RESULT = res
    out = np.concatenate([res.results[k]["out"] for k in range(N_CORES)], axis=0)
    return out.astype(np.float32)
